# revision 1
# baseline (speedup 1.0000x reference)
"""Trainium2 Bass kernel: 3x3 SAME conv (64->128ch) + bias, double-tanh, min over
channels, for x[16,64,224,224] -> y[16,1,224,224].

Strategy
--------
- Data-parallel over batch: 16 images / 8 NeuronCores = 2 images per core.
  Same NEFF on every core, different input shard (no collectives).
- min_c tanh(tanh(v_c)) == tanh(tanh(min_c v_c)) (tanh is monotone), so the
  double tanh is applied only to the per-pixel channel-minimum.
- Conv as implicit GEMM with the *image patch stationary*: tiles of M=128
  consecutive pixels of the padded row-stream (row stride 226; the 2 pad cols
  per row produce garbage outputs that are dropped at extraction), accumulated
  into PSUM[128, 128oc] with matmuls lhsT=[K, 128 px], rhs=[K, 128 oc].
  M=128 keeps NumWeights==128 so the compiler enables Fast Weight Load.
  Output channels land on the PSUM free dim, so the channel-min is a native
  free-dim DVE reduction.
- K-packing, 9 taps -> 5 K=128 matmuls:
    * strip tile SS: partitions 0:64 = padded row r, 64:128 = row r+1
      -> (kh=0,kw) + (kh=1,kw) pairs, kw = 0,1,2          (3 matmuls)
    * strip tile S2: partitions 0:64 = padded row r+2, 64:128 = same row
      shifted one column -> (kh=2,kw=0) + (kh=2,kw=1) pair (1 matmul)
    * (kh=2,kw=2) single with zeroed lower weight rows     (1 matmul)
- Bias is seeded into each 8-tile / 2-bank PSUM group by two K=1 rank-1
  matmuls (ones stationary, bias streaming, start=True) before the conv
  matmuls accumulate; the DVE then only does reduce_min over the oc axis.
  Minima collect in a stage tile that is
  PE-transposed so pixels become the free dim, double-tanh'd on ScalarE, and
  DMA'd to a DRAM scratch in padded-stream order; one strided DRAM->DRAM DMA
  per image extracts the valid 224x224.
- Host-side prep (cheap numpy): zero-pad x to [.,64,228,226] bf16 (plus a
  column-pre-shifted copy so every strip DMA is one contiguous run per
  channel), pre-transpose the weights into the [128,128] rhs tiles, tile the
  bias to [128,8,128] f32.
"""

import numpy as np
import ml_dtypes

import concourse.bass as bass
import concourse.mybir as mybir
import concourse.tile as tile
from concourse import bacc
from concourse.bass_utils import run_bass_kernel_spmd
from concourse.masks import make_identity

N_CORES = 8
B = 16
BPC = B // N_CORES  # images per core
IC, OC = 64, 128
H = W = 224
PW = 226    # padded row width in the pixel stream (= xp/xs width)
PH = 228    # xp rows (row r = image row r-1; rows 0, 225, 226, 227 zero)
R = 28      # output rows per strip
M = 128     # pixels per matmul tile
NPOS = R + 3   # padded-row positions in the main strip (upper half)
NPOS2 = R + 1  # positions in the kh=2 strip
GTOT = H * PW  # padded-stream length per image (50624)
NT = -(-GTOT // M)  # tiles per image (396)
CH = 128    # stage chunk size (tiles per transpose)
F32 = mybir.dt.float32

DT = mybir.dt.bfloat16
DT_NP = ml_dtypes.bfloat16

# Fold the bias into the PE: the (kh=2,kw=2) matmul runs with K=65 — lhsT row
# 64 is constant 1.0 (from a third strip tile) and rhs row 64 is the bias.
# Kills the DVE tensor_tensor bias-add entirely.
BIAS_IN_PE = False
STRIP_BUFS = 3
# Add bias with K=1 rank-1 matmuls per PSUM group (ones stationary, bias
# streaming) instead of the DVE tensor_tensor add.
ONES_MM_BIAS = True
# Emit the (kh=2,kw=2) singles of tiles (q, q+4) as adjacent K=64 matmuls in
# disjoint PE row-groups (partitions 0:64 vs 64:128) so they run concurrently;
# they target different PSUM banks.  The q+4 tile's tap data comes from the
# column-shifted strip half at stream offset g+1.
PAIRED_SINGLES = False

_CACHE: dict = {}
LAST_RESULT = None  # BassKernelResults of the most recent run (for profiling)


def _strip_of(t):
    """Strip index owning tile t (by its first pixel row)."""
    return min((t * M) // PW // R, H // R - 1)


def _emit(nc: bass.Bass, tc: tile.TileContext, y, xp, xs, wv, ws, bm,
          n_img=BPC, n_strips=None, nrep=1):
    """Emit the per-core program.

    y  : [n_img, 1, 224, 224] f32   ExternalOutput
    xp : [n_img, 64, 228, 226] DT   padded input
    xs : [n_img, 64, 228, 226] DT   same, pre-shifted one column left
    wv : [3, 128, 128] DT   rhs tiles for the (kh=0 | kh=1) K-pairs, kw=0..2
    ws : [3, 128, 128] DT   ws[0]: (kh=2,kw=0 | kh=2,kw=1) pair;
                            ws[1]: (kh=2,kw=2) single, rows 64:128 zero;
                            ws[2]: (kh=2,kw=2) single in rows 64:128
    bm : [128, 8, 128] f32  bias broadcast to partitions and 8 bank slots
    """
    if n_strips is None:
        n_strips = H // R
    n_tiles = NT if n_strips == H // R else ((n_strips * R * PW) // M)
    with (
        tc.tile_pool(name="consts", bufs=1) as cpool,
        tc.tile_pool(name="strips", bufs=STRIP_BUFS) as spool,
        tc.tile_pool(name="strips2", bufs=STRIP_BUFS) as s2pool,
        tc.tile_pool(name="strips3", bufs=STRIP_BUFS) as s3pool,
        tc.tile_pool(name="stage", bufs=4) as stpool,
        tc.tile_pool(name="obuf", bufs=4) as opool,
        tc.tile_pool(name="dscratch", bufs=2, space="DRAM") as dpool,
        tc.tile_pool(name="cpsum", bufs=3, space="PSUM") as cpsum,
        tc.tile_pool(name="tpsum", bufs=2, space="PSUM") as tpsum,
    ):
        # ---- constants ----
        identity = cpool.tile([128, 128], F32)
        make_identity(nc, identity)
        wv_sb = cpool.tile([128, 3, 128], DT)
        nc.sync.dma_start(wv_sb[:], wv.rearrange("t k n -> k t n"))
        ws_sb = cpool.tile([128, 3, 128], DT)
        nc.sync.dma_start(ws_sb[:], ws.rearrange("t k n -> k t n"))
        if not BIAS_IN_PE:
            bias_mat = cpool.tile([128, 8, 128], F32)
            nc.sync.dma_start(bias_mat[:], bm)
            if ONES_MM_BIAS:
                ones1 = cpool.tile([1, 128], DT)
                nc.gpsimd.memset(ones1[:], 1.0)
                bias_rhs = cpool.tile([1, 4, 128], DT)
                nc.vector.tensor_copy(bias_rhs[:], bias_mat[0:1, 0:4, :])
        else:
            ones_row = cpool.tile([1, NPOS2, PW], DT)
            nc.gpsimd.memset(ones_row[:], 1.0)

        for b in [bb for _ in range(nrep) for bb in range(n_img)]:
            ypad = dpool.tile([NT * M], F32, name="ypad")
            stage_t = None
            psum_t = None
            ssf = s2f = None
            cur_strip = -1
            for t in range(n_tiles):
                s = _strip_of(t)
                if s != cur_strip:
                    cur_strip = s
                    h0 = s * R
                    # main strip: upper rows h0..h0+R+2, lower +1 row
                    ss = spool.tile([128, NPOS, PW], DT, name="ss")
                    nc.sync.dma_start(
                        ss[0:64], xp[b, :, h0:h0 + NPOS, 0:PW])
                    nc.sync.dma_start(
                        ss[64:128], xp[b, :, h0 + 1:h0 + NPOS + 1, 0:PW])
                    ssf = ss.rearrange("p a c -> p (a c)")
                    # kh=2 strip: upper rows h0+2..h0+R+2, lower +1 col
                    s2 = s2pool.tile([128, NPOS2, PW], DT, name="s2")
                    nc.sync.dma_start(
                        s2[0:64], xp[b, :, h0 + 2:h0 + 2 + NPOS2, 0:PW])
                    nc.sync.dma_start(
                        s2[64:128], xs[b, :, h0 + 2:h0 + 2 + NPOS2, :])
                    s2f = s2.rearrange("p a c -> p (a c)")
                    if BIAS_IN_PE:
                        # kh=2 rows again, plus a ones-row at partition 64
                        s3 = s3pool.tile([128, NPOS2, PW], DT, name="s3")
                        nc.sync.dma_start(
                            s3[0:64], xp[b, :, h0 + 2:h0 + 2 + NPOS2, 0:PW])
                        nc.sync.dma_start(s3[64:65], ones_row[:])
                        s3f = s3.rearrange("p a c -> p (a c)")

                g = t * M - (s * R) * PW  # strip-local stream offset
                q = t % 8
                if q == 0:
                    psum_t = cpsum.tile([M, 8, 128], F32, name="psum_t")
                    if not BIAS_IN_PE and ONES_MM_BIAS:
                        # seed both banks with the bias via rank-1 matmuls
                        pt_flat = psum_t.rearrange("p q n -> p (q n)")
                        ngrp = min(8, n_tiles - t)
                        for bk in range(0, ngrp, 4):
                            w4 = min(4, ngrp - bk) * 128
                            nc.tensor.matmul(
                                pt_flat[:, bk * 128: bk * 128 + w4],
                                ones1[:],
                                bias_rhs.rearrange("p q n -> p (q n)")
                                [:, 0:w4],
                                start=True, stop=False)
                if q == 0:
                    pend = []  # deferred singles: (q, g, s2f)
                seeded = (not BIAS_IN_PE) and ONES_MM_BIAS
                for kw in range(3):      # (kh=0 | kh=1) pairs
                    nc.tensor.matmul(
                        psum_t[:, q], ssf[:, g + kw: g + kw + M], wv_sb[:, kw],
                        start=(kw == 0 and not seeded), stop=False)
                nc.tensor.matmul(        # (kh=2, kw=0 | kw=1) pair
                    psum_t[:, q], s2f[:, g: g + M], ws_sb[:, 0],
                    start=False, stop=False)
                # with bias-seeding, a stop mid-bank would clear the whole
                # zero region: only the last tile per bank emits stop=True
                stop_here = (not seeded) or q in (3, 7) or t == n_tiles - 1
                if PAIRED_SINGLES and not BIAS_IN_PE:
                    pend.append((q, g, s2f))
                    if q == 7 or t == n_tiles - 1:
                        ngrp = q + 1
                        for i in range(4):
                            if i < ngrp:
                                qa, ga, fa = pend[i]
                                nc.tensor.matmul(
                                    psum_t[:, qa],
                                    fa[0:64, ga + 2: ga + 2 + M],
                                    ws_sb[0:64, 1], start=False,
                                    stop=(i == min(3, ngrp - 1)))
                            if i + 4 < ngrp:
                                qb, gb, fb = pend[i + 4]
                                nc.tensor.matmul(
                                    psum_t[:, qb],
                                    fb[64:128, gb + 1: gb + 1 + M],
                                    ws_sb[64:128, 2], start=False,
                                    stop=(i + 4 == ngrp - 1))
                elif BIAS_IN_PE:
                    # (kh=2, kw=2) single + rank-1 bias via the ones-row
                    nc.tensor.matmul(
                        psum_t[:, q], s3f[0:65, g + 2: g + 2 + M],
                        ws_sb[0:65, 1], start=False, stop=stop_here)
                else:
                    nc.tensor.matmul(        # (kh=2, kw=2) single
                        psum_t[:, q], s2f[:, g + 2: g + 2 + M], ws_sb[:, 1],
                        start=False, stop=stop_here)

                if t % CH == 0:
                    stage_t = stpool.tile([128, CH], F32, name="stage_t")
                if q == 7 or t == n_tiles - 1:
                    nq = q + 1
                    cc = (t - q) % CH
                    if not BIAS_IN_PE and ONES_MM_BIAS:
                        pass  # bias already seeded at group start
                    elif not BIAS_IN_PE:
                        pt_flat = psum_t.rearrange("p q n -> p (q n)")
                        nc.vector.tensor_tensor(
                            pt_flat[:, 0:nq * 128],
                            pt_flat[:, 0:nq * 128],
                            bias_mat.rearrange("p q n -> p (q n)")
                            [:, 0:nq * 128],
                            mybir.AluOpType.add)
                    nc.vector.tensor_reduce(
                        out=stage_t[:, cc:cc + nq],
                        in_=psum_t[:, 0:nq],
                        axis=mybir.AxisListType.X,
                        op=mybir.AluOpType.min)
                if t % CH == CH - 1 or t == n_tiles - 1:
                    # chunk done: transpose -> tanh -> tanh -> scratch DMA
                    j = t // CH
                    w = t % CH + 1  # columns written in this chunk
                    tp = tpsum.tile([CH, 128], F32, name="tp")
                    nc.tensor.transpose(tp[0:w, :], stage_t[:, 0:w], identity)
                    ob = opool.tile([CH, 128], F32, name="ob")
                    nc.scalar.activation(
                        ob[0:w, :], tp[0:w, :],
                        mybir.ActivationFunctionType.Tanh)
                    nc.scalar.activation(
                        ob[0:w, :], ob[0:w, :],
                        mybir.ActivationFunctionType.Tanh)
                    nc.sync.dma_start(
                        ypad.rearrange("(t p) -> t p", p=M)[j * CH:j * CH + w],
                        ob[0:w, :])
            # extract valid pixels: drop the 2 pad cols per padded row
            rows_out = (n_tiles * M) // PW  # complete rows (224 when full)
            nc.sync.dma_start(
                y[b, 0, 0:rows_out, :],
                ypad[0:GTOT].rearrange("(h c) -> h c", c=PW)[0:rows_out, 0:W])


def _build(n_img=BPC, n_strips=None, enable_asserts=False, nrep=1):
    # num_devices=1: pure data-parallel SPMD, no collectives — each core runs
    # an independent single-device NEFF on its own input shard.
    nc = bacc.Bacc(
        "TRN2",
        target_bir_lowering=False,
        debug=False,
        enable_asserts=enable_asserts,
        num_devices=1,
    )
    xp = nc.dram_tensor("xp", [n_img, IC, PH, PW], DT, kind="ExternalInput")
    xs = nc.dram_tensor("xs", [n_img, IC, PH, PW], DT, kind="ExternalInput")
    wv = nc.dram_tensor("wv", [3, 128, 128], DT, kind="ExternalInput")
    ws = nc.dram_tensor("ws", [3, 128, 128], DT, kind="ExternalInput")
    bm = (None if BIAS_IN_PE else
          nc.dram_tensor("bias_mat", [128, 8, 128], F32,
                         kind="ExternalInput"))
    y = nc.dram_tensor("y", [n_img, 1, H, W], F32, kind="ExternalOutput")
    with tile.TileContext(nc) as tc:
        _emit(nc, tc, y.ap(), xp.ap(), xs.ap(), wv.ap(), ws.ap(),
              bm.ap() if bm is not None else None,
              n_img=n_img, n_strips=n_strips, nrep=nrep)
    nc.compile()
    return nc


def prep_inputs(x, weight, bias):
    """Host-side layout prep (numpy only)."""
    x = np.asarray(x, dtype=np.float32)
    weight = np.asarray(weight, dtype=np.float32)
    bias = np.asarray(bias, dtype=np.float32)
    nb = x.shape[0]
    xpad = np.zeros((nb, IC, PH, PW + 1), dtype=np.float32)
    xpad[:, :, 1:225, 1:225] = x
    xpad = xpad.astype(DT_NP)
    xp = np.ascontiguousarray(xpad[:, :, :, 0:PW])
    xs = np.ascontiguousarray(xpad[:, :, :, 1:PW + 1])
    wv = np.zeros((3, 128, 128), dtype=np.float32)
    ws = np.zeros((3, 128, 128), dtype=np.float32)
    for kw in range(3):
        wv[kw, 0:64] = weight[:, :, 0, kw].T
        wv[kw, 64:128] = weight[:, :, 1, kw].T
    ws[0, 0:64] = weight[:, :, 2, 0].T
    ws[0, 64:128] = weight[:, :, 2, 1].T
    ws[1, 0:64] = weight[:, :, 2, 2].T
    ws[2, 64:128] = weight[:, :, 2, 2].T
    if BIAS_IN_PE:
        ws[1, 64] = bias
    wv = np.ascontiguousarray(wv.astype(DT_NP))
    ws = np.ascontiguousarray(ws.astype(DT_NP))
    bm = np.ascontiguousarray(
        np.broadcast_to(bias[None, None, :], (128, 8, 128)).astype(np.float32))
    return xp, xs, wv, ws, bm


def kernel(x, weight, bias):
    global LAST_RESULT
    xp, xs, wv, ws, bm = prep_inputs(x, weight, bias)
    if "nc" not in _CACHE:
        _CACHE["nc"] = _build()
    nc = _CACHE["nc"]
    in_maps = []
    for c in range(N_CORES):
        m = {
            "xp": np.ascontiguousarray(xp[c * BPC:(c + 1) * BPC]),
            "xs": np.ascontiguousarray(xs[c * BPC:(c + 1) * BPC]),
            "wv": wv,
            "ws": ws,
        }
        if not BIAS_IN_PE:
            m["bias_mat"] = bm
        in_maps.append(m)
    res = run_bass_kernel_spmd(nc, in_maps, core_ids=list(range(N_CORES)))
    LAST_RESULT = res
    y = np.concatenate([r["y"] for r in res.results], axis=0)
    return y



# revision 2
# speedup vs baseline: 1.5284x; 1.5284x over previous
"""Trainium2 Bass kernel: 3x3 SAME conv (64->128ch) + bias, double-tanh, min over
channels, for x[16,64,224,224] -> y[16,1,224,224].

Strategy
--------
- Data-parallel over batch: 16 images / 8 NeuronCores = 2 images per core.
  Same NEFF on every core, different input shard (no collectives).
- min_c tanh(tanh(v_c)) == tanh(tanh(min_c v_c)) (tanh is monotone), so the
  double tanh is applied only to the per-pixel channel-minimum.
- Conv as implicit GEMM with the *image patch stationary*: tiles of M=128
  consecutive pixels of the padded row-stream (row stride 226; the 2 pad cols
  per row produce garbage outputs that are dropped at extraction), accumulated
  into PSUM[128, 128oc] with matmuls lhsT=[K, 128 px], rhs=[K, 128 oc].
  M=128 keeps NumWeights==128 so the compiler enables Fast Weight Load.
  Output channels land on the PSUM free dim, so the channel-min is a native
  free-dim DVE reduction.
- K-packing, 9 taps -> 5 K=128 matmuls:
    * strip tile SS: partitions 0:64 = padded row r, 64:128 = row r+1
      -> (kh=0,kw) + (kh=1,kw) pairs, kw = 0,1,2          (3 matmuls)
    * strip tile S2: partitions 0:64 = padded row r+2, 64:128 = same row
      shifted one column -> (kh=2,kw=0) + (kh=2,kw=1) pair (1 matmul)
    * (kh=2,kw=2) single with zeroed lower weight rows     (1 matmul)
- Bias is seeded into each 8-tile / 2-bank PSUM group by two K=1 rank-1
  matmuls (ones stationary, bias streaming, start=True) before the conv
  matmuls accumulate; the DVE then only does reduce_min over the oc axis.
  Minima collect in a stage tile that is
  PE-transposed so pixels become the free dim, double-tanh'd on ScalarE, and
  DMA'd to a DRAM scratch in padded-stream order; one strided DRAM->DRAM DMA
  per image extracts the valid 224x224.
- Host-side prep (cheap numpy): zero-pad x to [.,64,228,226] bf16 (plus a
  column-pre-shifted copy so every strip DMA is one contiguous run per
  channel), pre-transpose the weights into the [128,128] rhs tiles, tile the
  bias to [128,8,128] f32.
"""

import numpy as np
import ml_dtypes

import concourse.bass as bass
import concourse.mybir as mybir
import concourse.tile as tile
from concourse import bacc
from concourse.bass_utils import run_bass_kernel_spmd
from concourse.masks import make_identity

N_CORES = 8
B = 16
BPC = B // N_CORES  # images per core
IC, OC = 64, 128
H = W = 224
PW = 226    # padded row width in the pixel stream (= xp/xs width)
PH = 228    # xp rows (row r = image row r-1; rows 0, 225, 226, 227 zero)
R = 28      # output rows per strip
M = 128     # pixels per matmul tile
NPOS = R + 3   # padded-row positions in the main strip (upper half)
NPOS2 = R + 1  # positions in the kh=2 strip
GTOT = H * PW  # padded-stream length per image (50624)
NT = -(-GTOT // M)  # tiles per image (396)
CH = 128    # stage chunk size (tiles per transpose)
F32 = mybir.dt.float32

DT = mybir.dt.bfloat16
DT_NP = ml_dtypes.bfloat16

# Fold the bias into the PE: the (kh=2,kw=2) matmul runs with K=65 — lhsT row
# 64 is constant 1.0 (from a third strip tile) and rhs row 64 is the bias.
# Kills the DVE tensor_tensor bias-add entirely.
BIAS_IN_PE = False
STRIP_BUFS = 3
# Add bias with K=1 rank-1 matmuls per PSUM group (ones stationary, bias
# streaming) instead of the DVE tensor_tensor add.
ONES_MM_BIAS = True
# Emit the (kh=2,kw=2) singles of tiles (q, q+4) as adjacent K=64 matmuls in
# disjoint PE row-groups (partitions 0:64 vs 64:128) so they run concurrently;
# they target different PSUM banks.  The q+4 tile's tap data comes from the
# column-shifted strip half at stream offset g+1.
PAIRED_SINGLES = False

_CACHE: dict = {}
LAST_RESULT = None  # BassKernelResults of the most recent run (for profiling)


def _strip_of(t):
    """Strip index owning tile t (by its first pixel row)."""
    return min((t * M) // PW // R, H // R - 1)


def _emit(nc: bass.Bass, tc: tile.TileContext, y, xp, xs, wv, ws, bm,
          n_img=BPC, n_strips=None, nrep=1):
    """Emit the per-core program.

    y  : [n_img, 1, 224, 224] f32   ExternalOutput
    xp : [n_img, 64, 228, 226] DT   padded input
    xs : [n_img, 64, 228, 226] DT   same, pre-shifted one column left
    wv : [3, 128, 128] DT   rhs tiles for the (kh=0 | kh=1) K-pairs, kw=0..2
    ws : [3, 128, 128] DT   ws[0]: (kh=2,kw=0 | kh=2,kw=1) pair;
                            ws[1]: (kh=2,kw=2) single, rows 64:128 zero;
                            ws[2]: (kh=2,kw=2) single in rows 64:128
    bm : [128, 8, 128] f32  bias broadcast to partitions and 8 bank slots
    """
    if n_strips is None:
        n_strips = H // R
    n_tiles = NT if n_strips == H // R else ((n_strips * R * PW) // M)
    with (
        tc.tile_pool(name="consts", bufs=1) as cpool,
        tc.tile_pool(name="strips", bufs=STRIP_BUFS) as spool,
        tc.tile_pool(name="strips2", bufs=STRIP_BUFS) as s2pool,
        tc.tile_pool(name="strips3", bufs=STRIP_BUFS) as s3pool,
        tc.tile_pool(name="stage", bufs=4) as stpool,
        tc.tile_pool(name="obuf", bufs=4) as opool,
        tc.tile_pool(name="dscratch", bufs=2, space="DRAM") as dpool,
        tc.tile_pool(name="cpsum", bufs=3, space="PSUM") as cpsum,
        tc.tile_pool(name="tpsum", bufs=2, space="PSUM") as tpsum,
    ):
        # ---- constants ----
        identity = cpool.tile([128, 128], F32)
        make_identity(nc, identity)
        wv_sb = cpool.tile([128, 3, 128], DT)
        nc.sync.dma_start(wv_sb[:], wv.rearrange("t k n -> k t n"))
        ws_sb = cpool.tile([128, 3, 128], DT)
        nc.sync.dma_start(ws_sb[:], ws.rearrange("t k n -> k t n"))
        if not BIAS_IN_PE:
            bias_mat = cpool.tile([128, 8, 128], F32)
            nc.sync.dma_start(bias_mat[:], bm)
            if ONES_MM_BIAS:
                ones1 = cpool.tile([1, 128], DT)
                nc.gpsimd.memset(ones1[:], 1.0)
                bias_rhs = cpool.tile([1, 4, 128], DT)
                nc.vector.tensor_copy(bias_rhs[:], bias_mat[0:1, 0:4, :])
        else:
            ones_row = cpool.tile([1, NPOS2, PW], DT)
            nc.gpsimd.memset(ones_row[:], 1.0)

        for b in [bb for _ in range(nrep) for bb in range(n_img)]:
            ypad = dpool.tile([NT * M], F32, name="ypad")
            stage_t = None
            psum_t = None
            ssf = s2f = None
            cur_strip = -1
            for t in range(n_tiles):
                s = _strip_of(t)
                if s != cur_strip:
                    cur_strip = s
                    h0 = s * R
                    # main strip: upper rows h0..h0+R+2, lower +1 row
                    ss = spool.tile([128, NPOS, PW], DT, name="ss")
                    nc.sync.dma_start(
                        ss[0:64], xp[b, :, h0:h0 + NPOS, 0:PW])
                    nc.sync.dma_start(
                        ss[64:128], xp[b, :, h0 + 1:h0 + NPOS + 1, 0:PW])
                    ssf = ss.rearrange("p a c -> p (a c)")
                    # kh=2 strip: upper rows h0+2..h0+R+2, lower +1 col
                    s2 = s2pool.tile([128, NPOS2, PW], DT, name="s2")
                    nc.sync.dma_start(
                        s2[0:64], xp[b, :, h0 + 2:h0 + 2 + NPOS2, 0:PW])
                    nc.sync.dma_start(
                        s2[64:128], xs[b, :, h0 + 2:h0 + 2 + NPOS2, :])
                    s2f = s2.rearrange("p a c -> p (a c)")
                    if BIAS_IN_PE:
                        # kh=2 rows again, plus a ones-row at partition 64
                        s3 = s3pool.tile([128, NPOS2, PW], DT, name="s3")
                        nc.sync.dma_start(
                            s3[0:64], xp[b, :, h0 + 2:h0 + 2 + NPOS2, 0:PW])
                        nc.sync.dma_start(s3[64:65], ones_row[:])
                        s3f = s3.rearrange("p a c -> p (a c)")

                g = t * M - (s * R) * PW  # strip-local stream offset
                q = t % 8
                if q == 0:
                    psum_t = cpsum.tile([M, 8, 128], F32, name="psum_t")
                    if not BIAS_IN_PE and ONES_MM_BIAS:
                        # seed both banks with the bias via rank-1 matmuls
                        pt_flat = psum_t.rearrange("p q n -> p (q n)")
                        ngrp = min(8, n_tiles - t)
                        for bk in range(0, ngrp, 4):
                            w4 = min(4, ngrp - bk) * 128
                            nc.tensor.matmul(
                                pt_flat[:, bk * 128: bk * 128 + w4],
                                ones1[:],
                                bias_rhs.rearrange("p q n -> p (q n)")
                                [:, 0:w4],
                                start=True, stop=False)
                if q == 0:
                    pend = []  # deferred singles: (q, g, s2f)
                seeded = (not BIAS_IN_PE) and ONES_MM_BIAS
                for kw in range(3):      # (kh=0 | kh=1) pairs
                    nc.tensor.matmul(
                        psum_t[:, q], ssf[:, g + kw: g + kw + M], wv_sb[:, kw],
                        start=(kw == 0 and not seeded), stop=False)
                nc.tensor.matmul(        # (kh=2, kw=0 | kw=1) pair
                    psum_t[:, q], s2f[:, g: g + M], ws_sb[:, 0],
                    start=False, stop=False)
                # with bias-seeding, a stop mid-bank would clear the whole
                # zero region: only the last tile per bank emits stop=True
                stop_here = (not seeded) or q in (3, 7) or t == n_tiles - 1
                if PAIRED_SINGLES and not BIAS_IN_PE:
                    pend.append((q, g, s2f))
                    if q == 7 or t == n_tiles - 1:
                        ngrp = q + 1
                        for i in range(4):
                            if i < ngrp:
                                qa, ga, fa = pend[i]
                                nc.tensor.matmul(
                                    psum_t[:, qa],
                                    fa[0:64, ga + 2: ga + 2 + M],
                                    ws_sb[0:64, 1], start=False,
                                    stop=(i == min(3, ngrp - 1)))
                            if i + 4 < ngrp:
                                qb, gb, fb = pend[i + 4]
                                nc.tensor.matmul(
                                    psum_t[:, qb],
                                    fb[64:128, gb + 1: gb + 1 + M],
                                    ws_sb[64:128, 2], start=False,
                                    stop=(i + 4 == ngrp - 1))
                elif BIAS_IN_PE:
                    # (kh=2, kw=2) single + rank-1 bias via the ones-row
                    nc.tensor.matmul(
                        psum_t[:, q], s3f[0:65, g + 2: g + 2 + M],
                        ws_sb[0:65, 1], start=False, stop=stop_here)
                else:
                    nc.tensor.matmul(        # (kh=2, kw=2) single
                        psum_t[:, q], s2f[:, g + 2: g + 2 + M], ws_sb[:, 1],
                        start=False, stop=stop_here)

                if t % CH == 0:
                    stage_t = stpool.tile([128, CH], F32, name="stage_t")
                if q == 7 or t == n_tiles - 1:
                    nq = q + 1
                    cc = (t - q) % CH
                    if not BIAS_IN_PE and ONES_MM_BIAS:
                        pass  # bias already seeded at group start
                    elif not BIAS_IN_PE:
                        pt_flat = psum_t.rearrange("p q n -> p (q n)")
                        nc.vector.tensor_tensor(
                            pt_flat[:, 0:nq * 128],
                            pt_flat[:, 0:nq * 128],
                            bias_mat.rearrange("p q n -> p (q n)")
                            [:, 0:nq * 128],
                            mybir.AluOpType.add)
                    nc.vector.tensor_reduce(
                        out=stage_t[:, cc:cc + nq],
                        in_=psum_t[:, 0:nq],
                        axis=mybir.AxisListType.X,
                        op=mybir.AluOpType.min)
                if t % CH == CH - 1 or t == n_tiles - 1:
                    # chunk done: transpose -> tanh -> tanh -> scratch DMA
                    j = t // CH
                    w = t % CH + 1  # columns written in this chunk
                    tp = tpsum.tile([CH, 128], F32, name="tp")
                    nc.tensor.transpose(tp[0:w, :], stage_t[:, 0:w], identity)
                    ob = opool.tile([CH, 128], F32, name="ob")
                    nc.scalar.activation(
                        ob[0:w, :], tp[0:w, :],
                        mybir.ActivationFunctionType.Tanh)
                    nc.scalar.activation(
                        ob[0:w, :], ob[0:w, :],
                        mybir.ActivationFunctionType.Tanh)
                    nc.sync.dma_start(
                        ypad.rearrange("(t p) -> t p", p=M)[j * CH:j * CH + w],
                        ob[0:w, :])
            # extract valid pixels: drop the 2 pad cols per padded row
            rows_out = (n_tiles * M) // PW  # complete rows (224 when full)
            nc.sync.dma_start(
                y[b, 0, 0:rows_out, :],
                ypad[0:GTOT].rearrange("(h c) -> h c", c=PW)[0:rows_out, 0:W])


def _build(n_img=BPC, n_strips=None, enable_asserts=False, nrep=1):
    # num_devices=1: pure data-parallel SPMD, no collectives — each core runs
    # an independent single-device NEFF on its own input shard.
    nc = bacc.Bacc(
        "TRN2",
        target_bir_lowering=False,
        debug=False,
        enable_asserts=enable_asserts,
        num_devices=1,
    )
    xp = nc.dram_tensor("xp", [n_img, IC, PH, PW], DT, kind="ExternalInput")
    xs = nc.dram_tensor("xs", [n_img, IC, PH, PW], DT, kind="ExternalInput")
    wv = nc.dram_tensor("wv", [3, 128, 128], DT, kind="ExternalInput")
    ws = nc.dram_tensor("ws", [3, 128, 128], DT, kind="ExternalInput")
    bm = (None if BIAS_IN_PE else
          nc.dram_tensor("bias_mat", [128, 8, 128], F32,
                         kind="ExternalInput"))
    y = nc.dram_tensor("y", [n_img, 1, H, W], F32, kind="ExternalOutput")
    with tile.TileContext(nc) as tc:
        _emit(nc, tc, y.ap(), xp.ap(), xs.ap(), wv.ap(), ws.ap(),
              bm.ap() if bm is not None else None,
              n_img=n_img, n_strips=n_strips, nrep=nrep)
    nc.compile()
    return nc


def prep_inputs(x, weight, bias):
    """Host-side layout prep (numpy only)."""
    x = np.asarray(x, dtype=np.float32)
    weight = np.asarray(weight, dtype=np.float32)
    bias = np.asarray(bias, dtype=np.float32)
    nb = x.shape[0]
    xpad = np.zeros((nb, IC, PH, PW + 1), dtype=np.float32)
    xpad[:, :, 1:225, 1:225] = x
    xpad = xpad.astype(DT_NP)
    xp = np.ascontiguousarray(xpad[:, :, :, 0:PW])
    xs = np.ascontiguousarray(xpad[:, :, :, 1:PW + 1])
    wv = np.zeros((3, 128, 128), dtype=np.float32)
    ws = np.zeros((3, 128, 128), dtype=np.float32)
    for kw in range(3):
        wv[kw, 0:64] = weight[:, :, 0, kw].T
        wv[kw, 64:128] = weight[:, :, 1, kw].T
    ws[0, 0:64] = weight[:, :, 2, 0].T
    ws[0, 64:128] = weight[:, :, 2, 1].T
    ws[1, 0:64] = weight[:, :, 2, 2].T
    ws[2, 64:128] = weight[:, :, 2, 2].T
    if BIAS_IN_PE:
        ws[1, 64] = bias
    wv = np.ascontiguousarray(wv.astype(DT_NP))
    ws = np.ascontiguousarray(ws.astype(DT_NP))
    bm = np.ascontiguousarray(
        np.broadcast_to(bias[None, None, :], (128, 8, 128)).astype(np.float32))
    return xp, xs, wv, ws, bm


def make_in_maps(x, weight, bias):
    xp, xs, wv, ws, bm = prep_inputs(x, weight, bias)
    in_maps = []
    for c in range(N_CORES):
        m = {
            "xp": np.ascontiguousarray(xp[c * BPC:(c + 1) * BPC]),
            "xs": np.ascontiguousarray(xs[c * BPC:(c + 1) * BPC]),
            "wv": wv,
            "ws": ws,
        }
        if not BIAS_IN_PE:
            m["bias_mat"] = bm
        in_maps.append(m)
    return in_maps


def kernel(x, weight, bias):
    global LAST_RESULT
    if "nc" not in _CACHE:
        _CACHE["nc"] = _build()
    nc = _CACHE["nc"]
    in_maps = make_in_maps(x, weight, bias)
    res = run_bass_kernel_spmd(nc, in_maps, core_ids=list(range(N_CORES)))
    LAST_RESULT = res
    y = np.concatenate([r["y"] for r in res.results], axis=0)
    return y



# revision 7
# speedup vs baseline: 1.8814x; 1.2310x over previous
"""Trainium2 Bass kernel: 3x3 SAME conv (64->128ch) + bias, double-tanh, min over
channels, for x[16,64,224,224] -> y[16,1,224,224].

Strategy
--------
- Data-parallel over batch: 16 images / 8 NeuronCores = 2 images per core.
  Same NEFF on every core, different input shard (no collectives).
- min_c tanh(tanh(v_c)) == tanh(tanh(min_c v_c)) (tanh is monotone), so the
  double tanh is applied only to the per-pixel channel-minimum.
- Conv as implicit GEMM with the *image patch stationary*: tiles of M=128
  consecutive pixels of the padded row-stream (row stride 226; the 2 pad cols
  per row produce garbage outputs that are dropped at extraction), accumulated
  into PSUM[128, 128oc] with matmuls lhsT=[K, 128 px], rhs=[K, 128 oc].
  M=128 keeps NumWeights==128 so the compiler enables Fast Weight Load.
  Output channels land on the PSUM free dim, so the channel-min is a native
  free-dim DVE reduction.
- K-packing, 9 taps -> 5 K=128 matmuls:
    * strip tile SS: partitions 0:64 = padded row r, 64:128 = row r+1
      -> (kh=0,kw) + (kh=1,kw) pairs, kw = 0,1,2          (3 matmuls)
    * strip tile S2: partitions 0:64 = padded row r+2, 64:128 = same row
      shifted one column -> (kh=2,kw=0) + (kh=2,kw=1) pair (1 matmul)
    * (kh=2,kw=2) single with zeroed lower weight rows     (1 matmul)
- Bias is seeded into each 8-tile / 2-bank PSUM group by two K=1 rank-1
  matmuls (ones stationary, bias streaming, start=True) before the conv
  matmuls accumulate; the DVE then only does reduce_min over the oc axis.
  Minima collect in a stage tile that is
  PE-transposed so pixels become the free dim, double-tanh'd on ScalarE, and
  DMA'd to a DRAM scratch in padded-stream order; one strided DRAM->DRAM DMA
  per image extracts the valid 224x224.
- Host-side prep (cheap numpy): zero-pad x to [.,64,228,226] bf16 (plus a
  column-pre-shifted copy so every strip DMA is one contiguous run per
  channel), pre-transpose the weights into the [128,128] rhs tiles, tile the
  bias to [128,8,128] f32.
"""

import numpy as np
import ml_dtypes

import concourse.bass as bass
import concourse.mybir as mybir
import concourse.tile as tile
from concourse import bacc
from concourse.bass_utils import run_bass_kernel_spmd
from concourse.masks import make_identity

N_CORES = 8
B = 16
BPC = B // N_CORES  # images per core
IC, OC = 64, 128
H = W = 224
PW = 226    # padded row width in the pixel stream (= xp/xs width)
PH = 228    # xp rows (row r = image row r-1; rows 0, 225, 226, 227 zero)
R = 28      # output rows per strip
M = 128     # pixels per matmul tile
NPOS = R + 3   # padded-row positions in the main strip (upper half)
NPOS2 = R + 1  # positions in the kh=2 strip
GTOT = H * PW  # padded-stream length per image (50624)
NT = -(-GTOT // M)  # tiles per image (396)
CH = 128    # stage chunk size (tiles per transpose)
F32 = mybir.dt.float32

DT = mybir.dt.bfloat16
DT_NP = ml_dtypes.bfloat16

# Fold the bias into the PE: the (kh=2,kw=2) matmul runs with K=65 — lhsT row
# 64 is constant 1.0 (from a third strip tile) and rhs row 64 is the bias.
# Kills the DVE tensor_tensor bias-add entirely.
BIAS_IN_PE = False
STRIP_BUFS = 3
# Add bias with K=1 rank-1 matmuls per PSUM group (ones stationary, bias
# streaming) instead of the DVE tensor_tensor add.
ONES_MM_BIAS = True
# Emit the (kh=2,kw=2) singles of tiles (q, q+4) as adjacent K=64 matmuls in
# disjoint PE row-groups (partitions 0:64 vs 64:128) so they run concurrently;
# they target different PSUM banks.  The q+4 tile's tap data comes from the
# column-shifted strip half at stream offset g+1.
PAIRED_SINGLES = False

_CACHE: dict = {}
LAST_RESULT = None  # BassKernelResults of the most recent run (for profiling)


def _strip_of(t):
    """Strip index owning tile t (by its first pixel row)."""
    return min((t * M) // PW // R, H // R - 1)


def _emit(nc: bass.Bass, tc: tile.TileContext, y, xp, xs, wv, ws, bm,
          n_img=BPC, n_strips=None, nrep=1, loop_n=1):
    """Emit the per-core program.

    y  : [n_img, 1, 224, 224] f32   ExternalOutput
    xp : [n_img, 64, 228, 226] DT   padded input
    xs : [n_img, 64, 228, 226] DT   same, pre-shifted one column left
    wv : [3, 128, 128] DT   rhs tiles for the (kh=0 | kh=1) K-pairs, kw=0..2
    ws : [3, 128, 128] DT   ws[0]: (kh=2,kw=0 | kh=2,kw=1) pair;
                            ws[1]: (kh=2,kw=2) single, rows 64:128 zero;
                            ws[2]: (kh=2,kw=2) single in rows 64:128
    bm : [128, 8, 128] f32  bias broadcast to partitions and 8 bank slots
    """
    if n_strips is None:
        n_strips = H // R
    n_tiles = NT if n_strips == H // R else ((n_strips * R * PW) // M)
    with (
        tc.tile_pool(name="consts", bufs=1) as cpool,
        tc.tile_pool(name="strips", bufs=STRIP_BUFS) as spool,
        tc.tile_pool(name="strips2", bufs=STRIP_BUFS) as s2pool,
        tc.tile_pool(name="strips3", bufs=STRIP_BUFS) as s3pool,
        tc.tile_pool(name="stage", bufs=4) as stpool,
        tc.tile_pool(name="obuf", bufs=4) as opool,
        tc.tile_pool(name="dscratch", bufs=2, space="DRAM") as dpool,
        tc.tile_pool(name="cpsum", bufs=3, space="PSUM") as cpsum,
        tc.tile_pool(name="tpsum", bufs=2, space="PSUM") as tpsum,
    ):
        # ---- constants ----
        identity = cpool.tile([128, 128], F32)
        make_identity(nc, identity)
        wv_sb = cpool.tile([128, 3, 128], DT)
        nc.sync.dma_start(wv_sb[:], wv.rearrange("t k n -> k t n"))
        ws_sb = cpool.tile([128, 3, 128], DT)
        nc.sync.dma_start(ws_sb[:], ws.rearrange("t k n -> k t n"))
        if not BIAS_IN_PE:
            bias_mat = cpool.tile([128, 8, 128], F32)
            nc.sync.dma_start(bias_mat[:], bm)
            if ONES_MM_BIAS:
                ones1 = cpool.tile([1, 128], DT)
                nc.gpsimd.memset(ones1[:], 1.0)
                bias_rhs = cpool.tile([1, 4, 128], DT)
                nc.vector.tensor_copy(bias_rhs[:], bias_mat[0:1, 0:4, :])
        else:
            ones_row = cpool.tile([1, NPOS2, PW], DT)
            nc.gpsimd.memset(ones_row[:], 1.0)

        def _image_loop():
            for b in [bb for _ in range(nrep) for bb in range(n_img)]:
                _one_image(b)

        def _one_image(b):
            ypad = dpool.tile([NT * M], F32, name="ypad")
            stage_t = None
            psum_t = None
            ssf = s2f = None
            cur_strip = -1
            for t in range(n_tiles):
                s = _strip_of(t)
                if s != cur_strip:
                    cur_strip = s
                    h0 = s * R
                    # main strip: upper rows h0..h0+R+2, lower +1 row
                    ss = spool.tile([128, NPOS, PW], DT, name="ss")
                    nc.sync.dma_start(
                        ss[0:64], xp[b, :, h0:h0 + NPOS, 0:PW])
                    nc.sync.dma_start(
                        ss[64:128], xp[b, :, h0 + 1:h0 + NPOS + 1, 0:PW])
                    ssf = ss.rearrange("p a c -> p (a c)")
                    # kh=2 strip: upper rows h0+2..h0+R+2, lower +1 col
                    s2 = s2pool.tile([128, NPOS2, PW], DT, name="s2")
                    nc.sync.dma_start(
                        s2[0:64], xp[b, :, h0 + 2:h0 + 2 + NPOS2, 0:PW])
                    nc.sync.dma_start(
                        s2[64:128], xs[b, :, h0 + 2:h0 + 2 + NPOS2, :])
                    s2f = s2.rearrange("p a c -> p (a c)")
                    if BIAS_IN_PE:
                        # kh=2 rows again, plus a ones-row at partition 64
                        s3 = s3pool.tile([128, NPOS2, PW], DT, name="s3")
                        nc.sync.dma_start(
                            s3[0:64], xp[b, :, h0 + 2:h0 + 2 + NPOS2, 0:PW])
                        nc.sync.dma_start(s3[64:65], ones_row[:])
                        s3f = s3.rearrange("p a c -> p (a c)")

                g = t * M - (s * R) * PW  # strip-local stream offset
                q = t % 8
                if q == 0:
                    psum_t = cpsum.tile([M, 8, 128], F32, name="psum_t")
                    if not BIAS_IN_PE and ONES_MM_BIAS:
                        # seed both banks with the bias via rank-1 matmuls
                        pt_flat = psum_t.rearrange("p q n -> p (q n)")
                        ngrp = min(8, n_tiles - t)
                        for bk in range(0, ngrp, 4):
                            w4 = min(4, ngrp - bk) * 128
                            nc.tensor.matmul(
                                pt_flat[:, bk * 128: bk * 128 + w4],
                                ones1[:],
                                bias_rhs.rearrange("p q n -> p (q n)")
                                [:, 0:w4],
                                start=True, stop=False)
                if q == 0:
                    pend = []  # deferred singles: (q, g, s2f)
                seeded = (not BIAS_IN_PE) and ONES_MM_BIAS
                for kw in range(3):      # (kh=0 | kh=1) pairs
                    nc.tensor.matmul(
                        psum_t[:, q], ssf[:, g + kw: g + kw + M], wv_sb[:, kw],
                        start=(kw == 0 and not seeded), stop=False)
                nc.tensor.matmul(        # (kh=2, kw=0 | kw=1) pair
                    psum_t[:, q], s2f[:, g: g + M], ws_sb[:, 0],
                    start=False, stop=False)
                # with bias-seeding, a stop mid-bank would clear the whole
                # zero region: only the last tile per bank emits stop=True
                stop_here = (not seeded) or q in (3, 7) or t == n_tiles - 1
                if PAIRED_SINGLES and not BIAS_IN_PE:
                    pend.append((q, g, s2f))
                    if q == 7 or t == n_tiles - 1:
                        ngrp = q + 1
                        for i in range(4):
                            if i < ngrp:
                                qa, ga, fa = pend[i]
                                nc.tensor.matmul(
                                    psum_t[:, qa],
                                    fa[0:64, ga + 2: ga + 2 + M],
                                    ws_sb[0:64, 1], start=False,
                                    stop=(i == min(3, ngrp - 1)))
                            if i + 4 < ngrp:
                                qb, gb, fb = pend[i + 4]
                                nc.tensor.matmul(
                                    psum_t[:, qb],
                                    fb[64:128, gb + 1: gb + 1 + M],
                                    ws_sb[64:128, 2], start=False,
                                    stop=(i + 4 == ngrp - 1))
                elif BIAS_IN_PE:
                    # (kh=2, kw=2) single + rank-1 bias via the ones-row
                    nc.tensor.matmul(
                        psum_t[:, q], s3f[0:65, g + 2: g + 2 + M],
                        ws_sb[0:65, 1], start=False, stop=stop_here)
                else:
                    nc.tensor.matmul(        # (kh=2, kw=2) single
                        psum_t[:, q], s2f[:, g + 2: g + 2 + M], ws_sb[:, 1],
                        start=False, stop=stop_here)

                if t % CH == 0:
                    stage_t = stpool.tile([128, CH], F32, name="stage_t")
                if q == 7 or t == n_tiles - 1:
                    nq = q + 1
                    cc = (t - q) % CH
                    if not BIAS_IN_PE and ONES_MM_BIAS:
                        pass  # bias already seeded at group start
                    elif not BIAS_IN_PE:
                        pt_flat = psum_t.rearrange("p q n -> p (q n)")
                        nc.vector.tensor_tensor(
                            pt_flat[:, 0:nq * 128],
                            pt_flat[:, 0:nq * 128],
                            bias_mat.rearrange("p q n -> p (q n)")
                            [:, 0:nq * 128],
                            mybir.AluOpType.add)
                    nc.vector.tensor_reduce(
                        out=stage_t[:, cc:cc + nq],
                        in_=psum_t[:, 0:nq],
                        axis=mybir.AxisListType.X,
                        op=mybir.AluOpType.min)
                if t % CH == CH - 1 or t == n_tiles - 1:
                    # chunk done: transpose -> tanh -> tanh -> scratch DMA
                    j = t // CH
                    w = t % CH + 1  # columns written in this chunk
                    tp = tpsum.tile([CH, 128], F32, name="tp")
                    nc.tensor.transpose(tp[0:w, :], stage_t[:, 0:w], identity)
                    ob = opool.tile([CH, 128], F32, name="ob")
                    nc.scalar.activation(
                        ob[0:w, :], tp[0:w, :],
                        mybir.ActivationFunctionType.Tanh)
                    nc.scalar.activation(
                        ob[0:w, :], ob[0:w, :],
                        mybir.ActivationFunctionType.Tanh)
                    nc.sync.dma_start(
                        ypad.rearrange("(t p) -> t p", p=M)[j * CH:j * CH + w],
                        ob[0:w, :])
            # extract valid pixels: drop the 2 pad cols per padded row
            rows_out = (n_tiles * M) // PW  # complete rows (224 when full)
            nc.sync.dma_start(
                y[b, 0, 0:rows_out, :],
                ypad[0:GTOT].rearrange("(h c) -> h c", c=PW)[0:rows_out, 0:W])

        if loop_n > 1:
            with tc.For_i(0, loop_n):
                _image_loop()
        else:
            _image_loop()


def _build(n_img=BPC, n_strips=None, enable_asserts=False, nrep=1, loop_n=1):
    # num_devices=1: pure data-parallel SPMD, no collectives — each core runs
    # an independent single-device NEFF on its own input shard.
    nc = bacc.Bacc(
        "TRN2",
        target_bir_lowering=False,
        debug=False,
        enable_asserts=enable_asserts,
        num_devices=1,
    )
    xp = nc.dram_tensor("xp", [n_img, IC, PH, PW], DT, kind="ExternalInput")
    xs = nc.dram_tensor("xs", [n_img, IC, PH, PW], DT, kind="ExternalInput")
    wv = nc.dram_tensor("wv", [3, 128, 128], DT, kind="ExternalInput")
    ws = nc.dram_tensor("ws", [3, 128, 128], DT, kind="ExternalInput")
    bm = (None if BIAS_IN_PE else
          nc.dram_tensor("bias_mat", [128, 8, 128], F32,
                         kind="ExternalInput"))
    y = nc.dram_tensor("y", [n_img, 1, H, W], F32, kind="ExternalOutput")
    with tile.TileContext(nc) as tc:
        _emit(nc, tc, y.ap(), xp.ap(), xs.ap(), wv.ap(), ws.ap(),
              bm.ap() if bm is not None else None,
              n_img=n_img, n_strips=n_strips, nrep=nrep, loop_n=loop_n)
    nc.compile()
    return nc


def prep_inputs(x, weight, bias):
    """Host-side layout prep (numpy only)."""
    x = np.asarray(x, dtype=np.float32)
    weight = np.asarray(weight, dtype=np.float32)
    bias = np.asarray(bias, dtype=np.float32)
    nb = x.shape[0]
    xpad = np.zeros((nb, IC, PH, PW + 1), dtype=np.float32)
    xpad[:, :, 1:225, 1:225] = x
    xpad = xpad.astype(DT_NP)
    xp = np.ascontiguousarray(xpad[:, :, :, 0:PW])
    xs = np.ascontiguousarray(xpad[:, :, :, 1:PW + 1])
    wv = np.zeros((3, 128, 128), dtype=np.float32)
    ws = np.zeros((3, 128, 128), dtype=np.float32)
    for kw in range(3):
        wv[kw, 0:64] = weight[:, :, 0, kw].T
        wv[kw, 64:128] = weight[:, :, 1, kw].T
    ws[0, 0:64] = weight[:, :, 2, 0].T
    ws[0, 64:128] = weight[:, :, 2, 1].T
    ws[1, 0:64] = weight[:, :, 2, 2].T
    ws[2, 64:128] = weight[:, :, 2, 2].T
    if BIAS_IN_PE:
        ws[1, 64] = bias
    wv = np.ascontiguousarray(wv.astype(DT_NP))
    ws = np.ascontiguousarray(ws.astype(DT_NP))
    bm = np.ascontiguousarray(
        np.broadcast_to(bias[None, None, :], (128, 8, 128)).astype(np.float32))
    return xp, xs, wv, ws, bm


def make_in_maps(x, weight, bias):
    xp, xs, wv, ws, bm = prep_inputs(x, weight, bias)
    in_maps = []
    for c in range(N_CORES):
        m = {
            "xp": np.ascontiguousarray(xp[c * BPC:(c + 1) * BPC]),
            "xs": np.ascontiguousarray(xs[c * BPC:(c + 1) * BPC]),
            "wv": wv,
            "ws": ws,
        }
        if not BIAS_IN_PE:
            m["bias_mat"] = bm
        in_maps.append(m)
    return in_maps


def kernel(x, weight, bias):
    global LAST_RESULT
    if "nc" not in _CACHE:
        _CACHE["nc"] = _build()
    nc = _CACHE["nc"]
    in_maps = make_in_maps(x, weight, bias)
    res = run_bass_kernel_spmd(nc, in_maps, core_ids=list(range(N_CORES)))
    LAST_RESULT = res
    y = np.concatenate([r["y"] for r in res.results], axis=0)
    return y



# revision 8
# speedup vs baseline: 2.1303x; 1.1323x over previous
"""Trainium2 Bass kernel (fp8 DoubleRow variant): 3x3 SAME conv (64->128ch) +
bias, double-tanh, min over channels, x[16,64,224,224] -> y[16,1,224,224].

Same stream/strip/PSUM architecture as the bf16 baseline (see kernel.py), but
the 9-tap conv runs as 3 fp8 DoubleRow matmuls per 128-px tile (K_eff=256
each) instead of 5 bf16 K=128 matmuls:

  MM(kw):  k-tile0 = taps (kh=0,kw)|(kh=1,kw)   [partitions: row r | row r+1]
           k-tile1 = taps (kh=2,kw)|(kh=3,kw)   [kh=3 does not exist]
  The kh=3 half is repurposed: in "swi" mode the lower-half t=1 plane of the
  interleaved input is constant 1.0 and its weights are bias/192, so the three
  MMs inject the bias (64 partitions x 3 MMs x bias/192 = bias).  In "dr"
  mode that half has zero weights (garbage data) and bias is added on DVE.

Modes:
  MODE="swi": DoubleRowSwInterleave.  Host interleaves pairs
      XJ[row, 2c+t] = xpad[row + 2t, c]; the stationary AP is a contiguous
      256B window per partition, which keeps Fast Weight Load viable.  HW
      maps out partition m to pixel g+127-m (reversed); the chunk transpose
      uses an anti-identity so downstream layout is unchanged.
  MODE="dr": plain DoubleRow.  J tile holds two planes [128, 2, L]; k-tile
      step is the padded plane stride (16B-aligned).  No reversal.
"""

import os

import numpy as np
import ml_dtypes

import concourse.bass as bass
import concourse.mybir as mybir
import concourse.tile as tile
from concourse import bacc
from concourse.bass_utils import run_bass_kernel_spmd

MODE = "swi"   # "swi" | "dr" (swi: DoubleRowSwInterleave, 2x PE throughput)

N_CORES = 8
B = 16
BPC = B // N_CORES
IC, OC = 64, 128
H = W = 224
PW = 226      # padded row width in the pixel stream
PH = 228      # XJ rows (row r = image row r-1; rows 0,225..227 zero-ish)
R = 28        # output rows per strip
M = 128       # pixels per matmul tile
NPOS = R + 3  # padded-row positions per strip
L = NPOS * PW         # strip stream length (7006)
LP = -(-L // 16) * 16  # plane stride for "dr" mode (7008, 16B aligned)
GTOT = H * PW
NT = -(-GTOT // M)    # 396
CH = 128
F32 = mybir.dt.float32
DT = mybir.dt.float8e4
DT_NP = ml_dtypes.float8_e4m3

_CACHE: dict = {}
LAST_RESULT = None


def _strip_of(t):
    return min((t * M) // PW // R, H // R - 1)


def _emit(nc: bass.Bass, tc: tile.TileContext, y, xj, xj2, wq, bm, anti,
          n_img=BPC, nrep=1, loop_n=1):
    """Emit the per-core program.

    y   : [n_img, 1, 224, 224] f32  ExternalOutput
    xj  : [n_img, 64, PH, 2*PW] DT  interleaved (swi) / [n_img,64,PH,PW] (dr)
    xj2 : [n_img, 64, PH, 2*PW] DT  lower-half variant, t=1 plane = 1.0
          (swi only; None for dr)
    wq  : [3, 128, 2, 128] DT  DoubleRow rhs tiles per kw
    bm  : [128, 8, 128] f32  bias (dr mode only)
    anti: [128, 128] f32  anti-identity (swi) / identity (dr)
    """
    swi = MODE == "swi"
    with (
        tc.tile_pool(name="consts", bufs=1) as cpool,
        tc.tile_pool(name="strips", bufs=3) as spool,
        tc.tile_pool(name="stage", bufs=4) as stpool,
        tc.tile_pool(name="obuf", bufs=4) as opool,
        tc.tile_pool(name="dscratch", bufs=2, space="DRAM") as dpool,
        tc.tile_pool(name="cpsum", bufs=3, space="PSUM") as cpsum,
        tc.tile_pool(name="tpsum", bufs=2, space="PSUM") as tpsum,
    ):
        ident = cpool.tile([128, 128], F32)
        nc.sync.dma_start(ident[:], anti)
        wq_sb = cpool.tile([128, 3, 2, 128], DT)
        nc.sync.dma_start(wq_sb[:], wq.rearrange("t k two n -> k t two n"))
        if not swi:
            bias_mat = cpool.tile([128, 8, 128], F32)
            nc.sync.dma_start(bias_mat[:], bm)

        def _one_image(b):
            ypad = dpool.tile([NT * M], F32, name="ypad")
            stage_t = None
            psum_t = None
            jt = None
            cur_strip = -1
            for t in range(NT):
                s = _strip_of(t)
                if s != cur_strip:
                    cur_strip = s
                    h0 = s * R
                    if swi:
                        jt = spool.tile([128, L, 2], DT, name="jt")
                        jf = jt.rearrange("p i t -> p (i t)")
                        nc.sync.dma_start(
                            jf[0:64].rearrange("p (a c) -> p a c", a=NPOS),
                            xj[b, :, h0:h0 + NPOS, :])
                        nc.sync.dma_start(
                            jf[64:128].rearrange("p (a c) -> p a c", a=NPOS),
                            xj2[b, :, h0 + 1:h0 + NPOS + 1, :])
                    else:
                        jt = spool.tile([128, 2, LP], DT, name="jt")
                        # plane 0: rows h0+a (upper) / h0+1+a (lower)
                        nc.sync.dma_start(
                            jt[0:64, 0, 0:L].rearrange(
                                "p (a c) -> p a c", c=PW),
                            xj[b, :, h0:h0 + NPOS, :])
                        nc.sync.dma_start(
                            jt[64:128, 0, 0:L].rearrange(
                                "p (a c) -> p a c", c=PW),
                            xj[b, :, h0 + 1:h0 + NPOS + 1, :])
                        # plane 1: rows +2 (upper); lower half unused (zero w)
                        nc.sync.dma_start(
                            jt[0:64, 1, 0:L].rearrange(
                                "p (a c) -> p a c", c=PW),
                            xj[b, :, h0 + 2:h0 + NPOS + 2, :])

                g = t * M - (s * R) * PW
                q = t % 8
                if q == 0:
                    psum_t = cpsum.tile([M, 8, 128], F32, name="psum_t")
                for kw in range(3):
                    if swi:
                        lhsT = jt[:, g + kw:g + kw + M, :]
                        pm = mybir.MatmulPerfMode.DoubleRowSwInterleave
                    else:
                        lhsT = jt[:, :, g + kw:g + kw + M]
                        pm = mybir.MatmulPerfMode.DoubleRow
                    nc.tensor.matmul(
                        psum_t[:, q], lhsT, wq_sb[:, kw],
                        start=(kw == 0), stop=(kw == 2), perf_mode=pm)

                if t % CH == 0:
                    stage_t = stpool.tile([128, CH], F32, name="stage_t")
                if q == 7 or t == NT - 1:
                    nq = q + 1
                    cc = (t - q) % CH
                    if not swi:
                        pt_flat = psum_t.rearrange("p q n -> p (q n)")
                        nc.vector.tensor_tensor(
                            pt_flat[:, 0:nq * 128],
                            pt_flat[:, 0:nq * 128],
                            bias_mat.rearrange("p q n -> p (q n)")
                            [:, 0:nq * 128],
                            mybir.AluOpType.add)
                    nc.vector.tensor_reduce(
                        out=stage_t[:, cc:cc + nq],
                        in_=psum_t[:, 0:nq],
                        axis=mybir.AxisListType.X,
                        op=mybir.AluOpType.min)
                if t % CH == CH - 1 or t == NT - 1:
                    j = t // CH
                    w = t % CH + 1
                    tp = tpsum.tile([CH, 128], F32, name="tp")
                    nc.tensor.transpose(tp[0:w, :], stage_t[:, 0:w], ident)
                    ob = opool.tile([CH, 128], F32, name="ob")
                    nc.scalar.activation(
                        ob[0:w, :], tp[0:w, :],
                        mybir.ActivationFunctionType.Tanh)
                    nc.scalar.activation(
                        ob[0:w, :], ob[0:w, :],
                        mybir.ActivationFunctionType.Tanh)
                    nc.sync.dma_start(
                        ypad.rearrange("(t p) -> t p", p=M)[j * CH:j * CH + w],
                        ob[0:w, :])
            rows_out = (NT * M) // PW
            nc.sync.dma_start(
                y[b, 0, 0:rows_out, :],
                ypad[0:GTOT].rearrange("(h c) -> h c", c=PW)[0:rows_out, 0:W])

        def _image_loop():
            for b in [bb for _ in range(nrep) for bb in range(n_img)]:
                _one_image(b)

        if loop_n > 1:
            with tc.For_i(0, loop_n):
                _image_loop()
        else:
            _image_loop()


def _build(n_img=BPC, enable_asserts=False, nrep=1, loop_n=1):
    swi = MODE == "swi"
    nc = bacc.Bacc(
        "TRN2",
        target_bir_lowering=False,
        debug=False,
        enable_asserts=enable_asserts,
        num_devices=1,
    )
    wj = 2 * PW if swi else PW
    xj = nc.dram_tensor("xj", [n_img, IC, PH, wj], DT, kind="ExternalInput")
    xj2 = (nc.dram_tensor("xj2", [n_img, IC, PH, wj], DT,
                          kind="ExternalInput") if swi else None)
    wq = nc.dram_tensor("wq", [3, 128, 2, 128], DT, kind="ExternalInput")
    bm = (None if swi else
          nc.dram_tensor("bias_mat", [128, 8, 128], F32,
                         kind="ExternalInput"))
    anti = nc.dram_tensor("anti", [128, 128], F32, kind="ExternalInput")
    y = nc.dram_tensor("y", [n_img, 1, H, W], F32, kind="ExternalOutput")
    with tile.TileContext(nc) as tc:
        _emit(nc, tc, y.ap(), xj.ap(),
              xj2.ap() if xj2 is not None else None,
              wq.ap(), bm.ap() if bm is not None else None, anti.ap(),
              n_img=n_img, nrep=nrep, loop_n=loop_n)
    nc.compile()
    return nc


def prep_inputs(x, weight, bias):
    x = np.asarray(x, dtype=np.float32)
    weight = np.asarray(weight, dtype=np.float32)
    bias = np.asarray(bias, dtype=np.float32)
    nb = x.shape[0]
    swi = MODE == "swi"
    # padded input: rows 1..224 hold the image; 2 extra rows so row+2 exists
    xpad = np.zeros((nb, IC, PH + 2, PW), dtype=np.float32)
    xpad[:, :, 1:225, 1:225] = x
    x8 = np.clip(xpad, -240, 240).astype(DT_NP)
    if swi:
        xj = np.empty((nb, IC, PH, 2 * PW), dtype=DT_NP)
        xj[:, :, :, 0::2] = x8[:, :, 0:PH, :]
        xj[:, :, :, 1::2] = x8[:, :, 2:PH + 2, :]
        xj2 = np.empty((nb, IC, PH, 2 * PW), dtype=DT_NP)
        xj2[:, :, :, 0::2] = x8[:, :, 0:PH, :]
        xj2[:, :, :, 1::2] = np.asarray(1.0, DT_NP)
        xj2 = np.ascontiguousarray(xj2)
    else:
        xj = x8[:, :, 0:PH, :]
        xj2 = None
    xj = np.ascontiguousarray(xj)
    wq = np.zeros((3, 128, 2, 128), dtype=np.float32)
    for kw in range(3):
        wq[kw, 0:64, 0] = weight[:, :, 0, kw].T
        wq[kw, 64:128, 0] = weight[:, :, 1, kw].T
        wq[kw, 0:64, 1] = weight[:, :, 2, kw].T
        if swi:
            wq[kw, 64:128, 1] = bias[None, :] / 192.0
    wq = np.ascontiguousarray(np.clip(wq, -240, 240).astype(DT_NP))
    bm = np.ascontiguousarray(
        np.broadcast_to(bias[None, None, :], (128, 8, 128)).astype(np.float32))
    if swi:
        anti = np.zeros((128, 128), dtype=np.float32)
        anti[np.arange(128), 127 - np.arange(128)] = 1.0
    else:
        anti = np.eye(128, dtype=np.float32)
    return xj, xj2, wq, bm, np.ascontiguousarray(anti)


def make_in_maps(x, weight, bias):
    xj, xj2, wq, bm, anti = prep_inputs(x, weight, bias)
    in_maps = []
    for c in range(N_CORES):
        m = {
            "xj": np.ascontiguousarray(xj[c * BPC:(c + 1) * BPC]),
            "wq": wq,
            "anti": anti,
        }
        if MODE == "swi":
            m["xj2"] = np.ascontiguousarray(xj2[c * BPC:(c + 1) * BPC])
        else:
            m["bias_mat"] = bm
        in_maps.append(m)
    return in_maps


def kernel(x, weight, bias):
    global LAST_RESULT
    if "nc" not in _CACHE:
        _CACHE["nc"] = _build()
    nc = _CACHE["nc"]
    in_maps = make_in_maps(x, weight, bias)
    res = run_bass_kernel_spmd(nc, in_maps, core_ids=list(range(N_CORES)))
    LAST_RESULT = res
    y = np.concatenate([r["y"] for r in res.results], axis=0)
    return y
